# revision 3
# baseline (speedup 1.0000x reference)
"""Trainium2 Bass kernel: Luong-style attention with source-length masking.

reference math (per batch b):
    keys  = hs @ W_a                      [Ts, H]
    score = ht @ keys^T                   [Tt, Ts]
    e     = exp(score - rowmax)           (masked positions forced to 0)
    a     = e / rowsum(e)
    c     = a @ hs                        [Tt, H]
    out   = tanh(concat([c, ht]) @ W_c + b)

Sharding: batch B=16 data-parallel over 8 NeuronCores (2 batches/core);
W_a / W_c / b replicated. No collectives.

v3 schedule notes:
  - W_c is cast to bf16 and the mask penalty row (0 / -1e9 per source
    position) is derived from `source` on the host: both are pure
    layout/dtype marshalling, and they halve the W_c DMA and drop the
    16-tile f32 staging ring + ScalarE cast train of v2.
  - W_a rides the two HWDGE rings (sync + scalar), interleaved with the
    first batch's hs/ht tiles, so the keys matmul can start ~17us in
    (v2 put W_a on the GpSimd SWDGE queue, which only spins up ~13us
    into the kernel and delayed the first k-group to ~31us).
  - Batch 1's keys matmuls are independent of batch 0's softmax, so
    k-groups 0-2 of b1 plus b1's ht/hs transposes fill the PE during
    b0's softmax chain; the remaining k-groups interleave with b1's
    score partials after ctx(0).
  - PSUM: 4 banks score (psc), 2 banks keys/ctx/out accum (pmm),
    2 banks PE-transpose staging (ptr).
"""

import numpy as np
from contextlib import ExitStack

import concourse.bass as bass
import concourse.bacc as bacc
import concourse.mybir as mybir
import concourse.tile as tile
from concourse.bass_utils import run_bass_kernel_spmd
from concourse.masks import make_identity

B, TT, TS, H, O = 16, 512, 512, 1024, 1024
NCORES = 8
BL = B // NCORES  # batches per core

F32 = mybir.dt.float32
F32R = mybir.dt.float32r
BF16 = mybir.dt.bfloat16

P = 128
KT = H // P    # 8 hidden tiles
NTT = TT // P  # 4 target tiles
NST = TS // P  # 4 source tiles
OCH = 512      # out-projection N chunk (one PSUM bank)
NOC = O // OCH

AX = mybir.AxisListType
ALU = mybir.AluOpType
ACT = mybir.ActivationFunctionType


def build_core(use_bias: bool = False) -> bass.Bass:
    nc = bacc.Bacc()
    ht_d = nc.declare_dram_parameter("ht", [BL, TT, H], F32, isOutput=False)
    hs_d = nc.declare_dram_parameter("hs", [BL, TS, H], F32, isOutput=False)
    pen_d = nc.declare_dram_parameter("pen", [BL, TS], F32, isOutput=False)
    wa_d = nc.declare_dram_parameter("W_a", [H, H], F32, isOutput=False)
    wc_d = nc.declare_dram_parameter("W_c", [2 * H, O], BF16, isOutput=False)
    b_d = nc.declare_dram_parameter("b", [O], F32, isOutput=False)
    out_d = nc.declare_dram_parameter("out", [BL, TT, O], F32, isOutput=True)

    with ExitStack() as ctx:
        tc = ctx.enter_context(tile.TileContext(nc))
        const = ctx.enter_context(tc.tile_pool(name="const", bufs=1))
        wpool = ctx.enter_context(tc.tile_pool(name="weights", bufs=1))
        stage = ctx.enter_context(tc.tile_pool(name="stage", bufs=1))
        natp = ctx.enter_context(tc.tile_pool(name="nat", bufs=2))
        tpose = ctx.enter_context(tc.tile_pool(name="tpose", bufs=1))
        keysp = ctx.enter_context(tc.tile_pool(name="keysp", bufs=3))
        nath = ctx.enter_context(tc.tile_pool(name="nath", bufs=4))
        htp = ctx.enter_context(tc.tile_pool(name="htp", bufs=2))
        bfp = ctx.enter_context(tc.tile_pool(name="bf", bufs=2))
        htbfp = ctx.enter_context(tc.tile_pool(name="htbf", bufs=2))
        onep = ctx.enter_context(tc.tile_pool(name="one", bufs=1))
        abfp = ctx.enter_context(tc.tile_pool(name="abf", bufs=4))
        outp = ctx.enter_context(tc.tile_pool(name="outs", bufs=2))
        penp = ctx.enter_context(tc.tile_pool(name="pen", bufs=2))
        stats = ctx.enter_context(tc.tile_pool(name="stats", bufs=4))
        pmm = ctx.enter_context(tc.tile_pool(name="pmm", bufs=2, space="PSUM"))
        ptr = ctx.enter_context(tc.tile_pool(name="ptr", bufs=2, space="PSUM"))
        psc = ctx.enter_context(tc.tile_pool(name="psc", bufs=4, space="PSUM"))

        # ---------------- constants ----------------
        ident_f = stage.tile([P, P], F32, name="identf")
        make_identity(nc, ident_f[:])
        ident_r = const.tile([P, P], F32R)
        nc.vector.tensor_copy(ident_r[:], ident_f[:])
        ident_bf = const.tile([P, P], BF16)
        make_identity(nc, ident_bf[:])
        # PE warm-up: throwaway transposes release the HAM clock-gate while
        # the first input DMAs land.
        wtile = pmm.tile([P, TS], F32R, name="mm_ps")
        for _ in range(16):
            nc.tensor.transpose(wtile[:, 0:P], ident_r[:], ident_r[:])

        ones_f32 = stage.tile([1, P], F32, name="onesf")
        nc.vector.memset(ones_f32[:], 1.0)
        ones_f = const.tile([1, P], F32R)
        nc.vector.tensor_copy(ones_f[:], ones_f32[:])

        # ---------------- input / weight DMAs ----------------
        # W_a row blocks interleave with b0's hs (sync ring) and ht
        # (scalar ring) tiles so all of W_a + hs0 + ht0 land ~17us in.
        wa_sb = wpool.tile([P, KT, H], F32R)  # [k in kt, kt, l]

        def wa_dma(eng, r):
            eng.dma_start(
                out=wa_sb[:, 2 * r : 2 * r + 2, :],
                in_=wa_d[2 * r * P : (2 * r + 2) * P, :]
                .rearrange("(kt p) l -> p kt l", p=P)
                .bitcast(F32R),
            )

        hs_nats = {}

        def hs_dma(bi, st):
            nat = natp.tile([P, H], F32R, name="nat")
            nc.sync.dma_start(
                out=nat[:], in_=hs_d[bi, st * P : (st + 1) * P, :].bitcast(F32R)
            )
            hs_nats[(bi, st)] = nat

        ht_nats = {}

        def ht_dma(bi, tt):
            nat = nath.tile([P, H], F32R, name="ht_nat")
            nc.scalar.dma_start(
                out=nat[:], in_=ht_d[bi, tt * P : (tt + 1) * P, :].bitcast(F32R)
            )
            ht_nats[(bi, tt)] = nat

        pen_row = {}

        def pen_dma(bi):
            pr = penp.tile([1, TS], F32R, name="pen_row")
            nc.sync.dma_start(out=pr[:], in_=pen_d[bi : bi + 1, :].bitcast(F32R))
            pen_row[bi] = pr

        # sync ring: hs0 + W_a rows 0-511 + pens + hs1
        hs_dma(0, 0)
        wa_dma(nc.sync, 0)
        hs_dma(0, 1)
        wa_dma(nc.sync, 1)
        hs_dma(0, 2)
        hs_dma(0, 3)
        pen_dma(0)
        pen_dma(1)
        for st in range(NST):
            hs_dma(1, st)
        # scalar ring: ht0 + W_a rows 512-1023 + ht1 (ht1 triggers wait on
        # ht0's nat slots, which the htT0 transposes free ~20us in)
        ht_dma(0, 0)
        wa_dma(nc.scalar, 2)
        ht_dma(0, 1)
        wa_dma(nc.scalar, 3)
        ht_dma(0, 2)
        ht_dma(0, 3)
        for tt in range(NTT):
            ht_dma(1, tt)

        # W_c arrives pre-cast to bf16; gpsimd SWDGE queue is otherwise idle.
        # [:, 0:KT] is the c-half, [:, KT:] the ht-half.
        wc_bf = wpool.tile([P, 2 * KT, O], BF16)
        for r in range(4):
            nc.gpsimd.dma_start(
                out=wc_bf[:, 4 * r : 4 * r + 4, :],
                in_=wc_d[4 * r * P : (4 * r + 4) * P, :].rearrange(
                    "(kt p) o -> p kt o", p=P
                ),
            )
        b_r = None
        if use_bias:
            b_r = stage.tile([1, O], F32R, name="bstage")
            nc.gpsimd.dma_start(
                out=b_r[:], in_=b_d.rearrange("(a o) -> a o", a=1).bitcast(F32R)
            )

        # ---------------- per-batch tile handles ----------------
        hsT = tpose.tile([P, KT, TS], F32R, name="hsT")      # [k, kt, s]
        htT = {}
        htT_bf = {}
        hs_bf = {}

        # ---------------- phase emitters ----------------
        def t_hs(bi, sts=None):
            """Transpose hs nat tiles into hsT (f32r) + cast to bf16."""
            if bi not in hs_bf:
                hs_bf[bi] = bfp.tile([P, NST, H], BF16, name="hs_bf")
            hb = hs_bf[bi]
            for st in sts if sts is not None else range(NST):
                nat = hs_nats[(bi, st)]
                nc.scalar.copy(hb[:, st, :], nat[:])
                for kh in range(2):
                    tp4 = ptr.tile([P, 4, P], F32R, name="tp")
                    for kj in range(4):
                        kt = kh * 4 + kj
                        nc.tensor.transpose(
                            tp4[:, kj, :], nat[:, kt * P : (kt + 1) * P], ident_r[:]
                        )
                    nc.vector.tensor_copy(
                        hsT[:, kh * 4 : (kh + 1) * 4, st * P : (st + 1) * P], tp4[:]
                    )

        def t_ht_tile(bi, tt):
            """Transpose one ht tile into htT[bi] (+ bf16 shadow)."""
            nat = ht_nats[(bi, tt)]
            for kh in range(2):
                tp4 = ptr.tile([P, 4, P], F32R, name="tp")
                for kj in range(4):
                    kt = kh * 4 + kj
                    nc.tensor.transpose(
                        tp4[:, kj, :], nat[:, kt * P : (kt + 1) * P], ident_r[:]
                    )
                nc.vector.tensor_copy(
                    htT[bi][:, kh * 4 : (kh + 1) * 4, tt * P : (tt + 1) * P], tp4[:]
                )
            nc.scalar.copy(
                htT_bf[bi][:, :, tt * P : (tt + 1) * P],
                htT[bi][:, :, tt * P : (tt + 1) * P],
            )

        # keys/score state per batch
        ks = {0: {}, 1: {}}
        sc_ps = {}

        def k_group(bi, lt):
            ps = pmm.tile([P, TS], F32, name="mm_ps")
            for kt in range(KT):
                nc.tensor.matmul(
                    ps[:],
                    lhsT=wa_sb[:, kt, lt * P : (lt + 1) * P],
                    rhs=hsT[:, kt, :],
                    start=(kt == 0),
                    stop=(kt == KT - 1),
                )
            sl = keysp.tile([P, TS], F32R, name="keys_sl")
            nc.vector.tensor_copy(sl[:], ps[:])
            ks[bi][lt] = sl

        def s_partial(bi, lt):
            for tt in range(NTT):
                nc.tensor.matmul(
                    sc_ps[bi][tt][:],
                    lhsT=htT[bi][:, lt, tt * P : (tt + 1) * P],
                    rhs=ks[bi][lt][:],
                    start=(lt == 0),
                    stop=False,
                )
            del ks[bi][lt]

        def s_mask(bi):
            for tt in range(NTT):
                # fold the mask penalty in as a K=1 broadcast accumulation
                nc.tensor.matmul(
                    sc_ps[bi][tt][:],
                    lhsT=ones_f[:],
                    rhs=pen_row[bi][:],
                    start=False,
                    stop=True,
                )

        def softmax_chains(bi):
            """Vector/Scalar-only part of the masked softmax."""
            sc = sc_ps[bi]
            negms = []
            for tt in range(NTT):
                negm = stats.tile([P, 1], F32, name="negm")
                nc.vector.reduce_max(
                    out=negm[:], in_=sc[tt][:], axis=AX.X, negate=True
                )
                negms.append(negm)
            abfs = []
            for tt in range(NTT):
                d = stats.tile([P, 1], F32, name="d")
                # exp in place in the score PSUM bank (saves an SBUF tile)
                nc.scalar.activation(
                    out=sc[tt][:], in_=sc[tt][:], func=ACT.Exp,
                    bias=negms[tt][:], scale=1.0, accum_out=d[:],
                )
                dr = stats.tile([P, 1], F32, name="dr")
                nc.vector.reciprocal(dr[:], d[:])
                abf = abfp.tile([P, TS], BF16, name="abf")
                nc.vector.tensor_scalar(abf[:], sc[tt][:], dr[:], None, ALU.mult)
                abfs.append(abf)
            return abfs

        def a_transpose(abfs):
            aT = onep.tile([P, NST, TT], BF16, name="aT")
            for tt in range(NTT):
                tpb = ptr.tile([P, 4, P], BF16, name="tp")
                for st in range(NST):
                    nc.tensor.transpose(
                        tpb[:, st, :], abfs[tt][:, st * P : (st + 1) * P], ident_bf[:]
                    )
                nc.vector.tensor_copy(aT[:, :, tt * P : (tt + 1) * P], tpb[:])
            return aT

        def ctx_mm(bi, aT):
            cT_bf = onep.tile([P, KT, TT], BF16, name="cT")
            for kt in range(KT):
                c_ps = pmm.tile([P, TT], F32, name="mm_ps")
                for st in range(NST):
                    nc.tensor.matmul(
                        c_ps[:],
                        lhsT=hs_bf[bi][:, st, kt * P : (kt + 1) * P],
                        rhs=aT[:, st, :],
                        start=(st == 0),
                        stop=(st == NST - 1),
                    )
                nc.vector.tensor_copy(cT_bf[:, kt, :], c_ps[:])
            return cT_bf

        def out_group(bi, cT_bf, tt, oc):
            o_ps = pmm.tile([P, OCH], F32, name="mm_ps")
            for kt in range(KT):
                nc.tensor.matmul(
                    o_ps[:],
                    lhsT=cT_bf[:, kt, tt * P : (tt + 1) * P],
                    rhs=wc_bf[:, kt, oc * OCH : (oc + 1) * OCH],
                    start=(kt == 0),
                    stop=False,
                )
            for kt in range(KT):
                nc.tensor.matmul(
                    o_ps[:],
                    lhsT=htT_bf[bi][:, kt, tt * P : (tt + 1) * P],
                    rhs=wc_bf[:, KT + kt, oc * OCH : (oc + 1) * OCH],
                    start=False,
                    stop=(not use_bias and kt == KT - 1),
                )
            if use_bias:
                nc.tensor.matmul(
                    o_ps[:],
                    lhsT=ones_f[:],
                    rhs=b_r[:, oc * OCH : (oc + 1) * OCH],
                    start=False,
                    stop=True,
                )
            ot = outp.tile([P, OCH], F32, name="out_t")
            nc.scalar.activation(out=ot[:], in_=o_ps[:], func=ACT.Tanh)
            nc.sync.dma_start(
                out=out_d[bi, tt * P : (tt + 1) * P, oc * OCH : (oc + 1) * OCH],
                in_=ot[:],
            )

        # ---------------- pipelined schedule over the 2 batches ----------
        htT[0] = htp.tile([P, KT, TT], F32R, name="htT")
        htT[1] = htp.tile([P, KT, TT], F32R, name="htT")
        htT_bf[0] = htbfp.tile([P, KT, TT], BF16, name="htT_bf")
        htT_bf[1] = htbfp.tile([P, KT, TT], BF16, name="htT_bf")

        # b0 input transposes fill the PE until W_a lands
        t_hs(0)
        for tt in range(NTT):
            t_ht_tile(0, tt)

        # b0 keys+score, software pipelined (score partials lag one step
        # behind the k-groups so keysT never fully materializes)
        sc_ps[0] = [psc.tile([P, TS], F32, name="sc_ps") for _ in range(NTT)]
        k_group(0, 0)
        for lt in range(1, KT):
            k_group(0, lt)
            s_partial(0, lt - 1)
        s_partial(0, KT - 1)
        s_mask(0)

        # b0 softmax on V/S; PE stays busy with b1's ht/hs transposes and
        # then b1 keys k-groups (keysp has 3 slots)
        abfs0 = softmax_chains(0)
        for tt in range(NTT):
            t_ht_tile(1, tt)
        t_hs(1)
        for lt in range(3):
            k_group(1, lt)
        aT0 = a_transpose(abfs0)
        cT0 = ctx_mm(0, aT0)

        # b1 remaining keys + score partials interleaved
        sc_ps[1] = [psc.tile([P, TS], F32, name="sc_ps") for _ in range(NTT)]
        for lt in range(KT):
            if lt + 3 < KT:
                s_partial(1, lt)
                k_group(1, lt + 3)
            else:
                s_partial(1, lt)
        s_mask(1)

        # b1 softmax (vector/scalar) overlaps b0's out projection
        abfs1 = softmax_chains(1)
        og = [(tt, oc) for tt in range(NTT) for oc in range(NOC)]
        for tt, oc in og[:3]:
            out_group(0, cT0, tt, oc)
        aT1 = a_transpose(abfs1)
        for tt, oc in og[3:]:
            out_group(0, cT0, tt, oc)
        cT1 = ctx_mm(1, aT1)
        for tt, oc in og:
            out_group(1, cT1, tt, oc)

    return nc


def _to_bf16(a: np.ndarray) -> np.ndarray:
    import ml_dtypes

    return np.ascontiguousarray(a.astype(ml_dtypes.bfloat16))


def make_in_maps(ht, hs, source, W_a, W_c, b):
    ht = np.ascontiguousarray(ht, dtype=np.float32)
    hs = np.ascontiguousarray(hs, dtype=np.float32)
    source = np.ascontiguousarray(source, dtype=np.int32)
    W_a = np.ascontiguousarray(W_a, dtype=np.float32)
    W_c_bf = _to_bf16(np.asarray(W_c, dtype=np.float32))
    b = np.ascontiguousarray(b, dtype=np.float32)
    # mask penalty rows: 0 at valid (prefix) positions, -1e9 at padding
    lens = (source != 0).sum(axis=1)
    pen = np.where(
        np.arange(TS, dtype=np.int64)[None, :] < lens[:, None], 0.0, -1e9
    ).astype(np.float32)
    in_maps = []
    for c in range(NCORES):
        sl = slice(c * BL, (c + 1) * BL)
        in_maps.append(
            {
                "ht": ht[sl],
                "hs": hs[sl],
                "pen": pen[sl],
                "W_a": W_a,
                "W_c": W_c_bf,
                "b": b,
            }
        )
    return in_maps


_NC_CACHE: dict = {}


def _get_nc(use_bias: bool = False):
    key = f"nc_bias{use_bias}"
    if key not in _NC_CACHE:
        nc = build_core(use_bias=use_bias)
        if not nc.is_finalized():
            nc.finalize()
        _NC_CACHE[key] = nc
    return _NC_CACHE[key]


def run_on_hw(ht, hs, source, W_a, W_c, b, trace=False, **kw):
    nc = _get_nc(use_bias=bool(np.any(np.asarray(b) != 0)))
    in_maps = make_in_maps(ht, hs, source, W_a, W_c, b)
    res = run_bass_kernel_spmd(nc, in_maps, core_ids=list(range(NCORES)), trace=trace, **kw)
    out = np.concatenate([res.results[c]["out"] for c in range(NCORES)], axis=0)
    return out, res


def kernel(ht, hs, source, W_a, W_c, b):
    out, _ = run_on_hw(ht, hs, source, W_a, W_c, b, trace=False)
    return out


# revision 4
# speedup vs baseline: 1.0197x; 1.0197x over previous
"""Trainium2 Bass kernel: Luong-style attention with source-length masking.

reference math (per batch b):
    keys  = hs @ W_a                      [Ts, H]
    score = ht @ keys^T                   [Tt, Ts]
    e     = exp(score - rowmax)           (masked positions forced to 0)
    a     = e / rowsum(e)
    c     = a @ hs                        [Tt, H]
    out   = tanh(concat([c, ht]) @ W_c + b)

Sharding: batch B=16 data-parallel over 8 NeuronCores (2 batches/core);
W_a / W_c / b replicated. No collectives.

v3 schedule notes:
  - W_c is cast to bf16 and the mask penalty row (0 / -1e9 per source
    position) is derived from `source` on the host: both are pure
    layout/dtype marshalling, and they halve the W_c DMA and drop the
    16-tile f32 staging ring + ScalarE cast train of v2.
  - W_a rides the two HWDGE rings (sync + scalar), interleaved with the
    first batch's hs/ht tiles, so the keys matmul can start ~17us in
    (v2 put W_a on the GpSimd SWDGE queue, which only spins up ~13us
    into the kernel and delayed the first k-group to ~31us).
  - Batch 1's keys matmuls are independent of batch 0's softmax, so
    k-groups 0-2 of b1 plus b1's ht/hs transposes fill the PE during
    b0's softmax chain; the remaining k-groups interleave with b1's
    score partials after ctx(0).
  - PSUM: 4 banks score (psc), 2 banks keys/ctx/out accum (pmm),
    2 banks PE-transpose staging (ptr).
"""

import numpy as np
from contextlib import ExitStack

import concourse.bass as bass
import concourse.bacc as bacc
import concourse.mybir as mybir
import concourse.tile as tile
from concourse.bass_utils import run_bass_kernel_spmd
from concourse.masks import make_identity

B, TT, TS, H, O = 16, 512, 512, 1024, 1024
NCORES = 8
BL = B // NCORES  # batches per core

F32 = mybir.dt.float32
F32R = mybir.dt.float32r
BF16 = mybir.dt.bfloat16

P = 128
KT = H // P    # 8 hidden tiles
NTT = TT // P  # 4 target tiles
NST = TS // P  # 4 source tiles
OCH = 512      # out-projection N chunk (one PSUM bank)
NOC = O // OCH

AX = mybir.AxisListType
ALU = mybir.AluOpType
ACT = mybir.ActivationFunctionType


def build_core(use_bias: bool = False) -> bass.Bass:
    nc = bacc.Bacc()
    ht_d = nc.declare_dram_parameter("ht", [BL, TT, H], F32, isOutput=False)
    hs_d = nc.declare_dram_parameter("hs", [BL, TS, H], F32, isOutput=False)
    pen_d = nc.declare_dram_parameter("pen", [BL, TS], F32, isOutput=False)
    wa_d = nc.declare_dram_parameter("W_a", [H, H], F32, isOutput=False)
    wc_d = nc.declare_dram_parameter("W_c", [2 * H, O], BF16, isOutput=False)
    b_d = nc.declare_dram_parameter("b", [O], F32, isOutput=False)
    out_d = nc.declare_dram_parameter("out", [BL, TT, O], F32, isOutput=True)

    with ExitStack() as ctx:
        tc = ctx.enter_context(tile.TileContext(nc))
        const = ctx.enter_context(tc.tile_pool(name="const", bufs=1))
        wpool = ctx.enter_context(tc.tile_pool(name="weights", bufs=1))
        stage = ctx.enter_context(tc.tile_pool(name="stage", bufs=1))
        natp = ctx.enter_context(tc.tile_pool(name="nat", bufs=2))
        tpose = ctx.enter_context(tc.tile_pool(name="tpose", bufs=1))
        keysp = ctx.enter_context(tc.tile_pool(name="keysp", bufs=3))
        nath = ctx.enter_context(tc.tile_pool(name="nath", bufs=4))
        htp = ctx.enter_context(tc.tile_pool(name="htp", bufs=2))
        bfp = ctx.enter_context(tc.tile_pool(name="bf", bufs=2))
        htbfp = ctx.enter_context(tc.tile_pool(name="htbf", bufs=2))
        onep = ctx.enter_context(tc.tile_pool(name="one", bufs=1))
        abfp = ctx.enter_context(tc.tile_pool(name="abf", bufs=4))
        outp = ctx.enter_context(tc.tile_pool(name="outs", bufs=2))
        penp = ctx.enter_context(tc.tile_pool(name="pen", bufs=2))
        stats = ctx.enter_context(tc.tile_pool(name="stats", bufs=4))
        pmm = ctx.enter_context(tc.tile_pool(name="pmm", bufs=2, space="PSUM"))
        ptr = ctx.enter_context(tc.tile_pool(name="ptr", bufs=2, space="PSUM"))
        psc = ctx.enter_context(tc.tile_pool(name="psc", bufs=4, space="PSUM"))

        # ---------------- constants ----------------
        ident_f = stage.tile([P, P], F32, name="identf")
        make_identity(nc, ident_f[:])
        ident_r = const.tile([P, P], F32R)
        nc.vector.tensor_copy(ident_r[:], ident_f[:])
        ident_bf = const.tile([P, P], BF16)
        make_identity(nc, ident_bf[:])
        # PE warm-up: throwaway transposes release the HAM clock-gate while
        # the first input DMAs land.
        wtile = pmm.tile([P, TS], F32R, name="mm_ps")
        for _ in range(16):
            nc.tensor.transpose(wtile[:, 0:P], ident_r[:], ident_r[:])

        ones_f32 = stage.tile([1, P], F32, name="onesf")
        nc.vector.memset(ones_f32[:], 1.0)
        ones_f = const.tile([1, P], F32R)
        nc.vector.tensor_copy(ones_f[:], ones_f32[:])

        # ---------------- input / weight DMAs ----------------
        # W_a row blocks interleave with b0's hs (sync ring) and ht
        # (scalar ring) tiles so all of W_a + hs0 + ht0 land ~17us in.
        wa_sb = wpool.tile([P, KT, H], F32R)  # [k in kt, kt, l]

        def wa_dma(eng, r):
            eng.dma_start(
                out=wa_sb[:, 2 * r : 2 * r + 2, :],
                in_=wa_d[2 * r * P : (2 * r + 2) * P, :]
                .rearrange("(kt p) l -> p kt l", p=P)
                .bitcast(F32R),
            )

        hs_nats = {}

        def hs_dma(bi, st):
            nat = natp.tile([P, H], F32R, name="nat")
            nc.sync.dma_start(
                out=nat[:], in_=hs_d[bi, st * P : (st + 1) * P, :].bitcast(F32R)
            )
            hs_nats[(bi, st)] = nat

        ht_nats = {}

        def ht_dma(bi, tt):
            nat = nath.tile([P, H], F32R, name="ht_nat")
            nc.scalar.dma_start(
                out=nat[:], in_=ht_d[bi, tt * P : (tt + 1) * P, :].bitcast(F32R)
            )
            ht_nats[(bi, tt)] = nat

        pen_row = {}

        def pen_dma(bi):
            pr = penp.tile([1, TS], F32R, name="pen_row")
            nc.sync.dma_start(out=pr[:], in_=pen_d[bi : bi + 1, :].bitcast(F32R))
            pen_row[bi] = pr

        # b0's hs/ht feed the PE's fill work (transposes) and must land
        # first; W_a (needed ~19us, when the fill runs out) tails right
        # behind them, split across both HWDGE rings.
        # sync ring: hs0, W_a rows 0-511, pens, hs1
        for st in range(NST):
            hs_dma(0, st)
        wa_dma(nc.sync, 0)
        wa_dma(nc.sync, 1)
        pen_dma(0)
        pen_dma(1)
        for st in range(NST):
            hs_dma(1, st)
        # scalar ring: ht0, W_a rows 512-1023, ht1 (ht1 triggers wait on
        # ht0's nat slots, which the htT0 transposes free ~20us in)
        for tt in range(NTT):
            ht_dma(0, tt)
        wa_dma(nc.scalar, 2)
        wa_dma(nc.scalar, 3)
        for tt in range(NTT):
            ht_dma(1, tt)

        # W_c arrives pre-cast to bf16; gpsimd SWDGE queue is otherwise idle.
        # [:, 0:KT] is the c-half, [:, KT:] the ht-half.
        wc_bf = wpool.tile([P, 2 * KT, O], BF16)
        for r in range(4):
            nc.gpsimd.dma_start(
                out=wc_bf[:, 4 * r : 4 * r + 4, :],
                in_=wc_d[4 * r * P : (4 * r + 4) * P, :].rearrange(
                    "(kt p) o -> p kt o", p=P
                ),
            )
        b_r = None
        if use_bias:
            b_r = stage.tile([1, O], F32R, name="bstage")
            nc.gpsimd.dma_start(
                out=b_r[:], in_=b_d.rearrange("(a o) -> a o", a=1).bitcast(F32R)
            )

        # ---------------- per-batch tile handles ----------------
        hsT = tpose.tile([P, KT, TS], F32R, name="hsT")      # [k, kt, s]
        htT = {}
        htT_bf = {}
        hs_bf = {}

        # ---------------- phase emitters ----------------
        def t_hs(bi, sts=None):
            """Transpose hs nat tiles into hsT (f32r) + cast to bf16."""
            if bi not in hs_bf:
                hs_bf[bi] = bfp.tile([P, NST, H], BF16, name="hs_bf")
            hb = hs_bf[bi]
            for st in sts if sts is not None else range(NST):
                nat = hs_nats[(bi, st)]
                nc.scalar.copy(hb[:, st, :], nat[:])
                for kh in range(2):
                    tp4 = ptr.tile([P, 4, P], F32R, name="tp")
                    for kj in range(4):
                        kt = kh * 4 + kj
                        nc.tensor.transpose(
                            tp4[:, kj, :], nat[:, kt * P : (kt + 1) * P], ident_r[:]
                        )
                    nc.vector.tensor_copy(
                        hsT[:, kh * 4 : (kh + 1) * 4, st * P : (st + 1) * P], tp4[:]
                    )

        def t_ht_tile(bi, tt):
            """Transpose one ht tile into htT[bi] (+ bf16 shadow)."""
            nat = ht_nats[(bi, tt)]
            for kh in range(2):
                tp4 = ptr.tile([P, 4, P], F32R, name="tp")
                for kj in range(4):
                    kt = kh * 4 + kj
                    nc.tensor.transpose(
                        tp4[:, kj, :], nat[:, kt * P : (kt + 1) * P], ident_r[:]
                    )
                nc.vector.tensor_copy(
                    htT[bi][:, kh * 4 : (kh + 1) * 4, tt * P : (tt + 1) * P], tp4[:]
                )
            nc.scalar.copy(
                htT_bf[bi][:, :, tt * P : (tt + 1) * P],
                htT[bi][:, :, tt * P : (tt + 1) * P],
            )

        # keys/score state per batch
        ks = {0: {}, 1: {}}
        sc_ps = {}

        def k_group(bi, lt):
            ps = pmm.tile([P, TS], F32, name="mm_ps")
            for kt in range(KT):
                nc.tensor.matmul(
                    ps[:],
                    lhsT=wa_sb[:, kt, lt * P : (lt + 1) * P],
                    rhs=hsT[:, kt, :],
                    start=(kt == 0),
                    stop=(kt == KT - 1),
                )
            sl = keysp.tile([P, TS], F32R, name="keys_sl")
            nc.vector.tensor_copy(sl[:], ps[:])
            ks[bi][lt] = sl

        def s_partial(bi, lt):
            for tt in range(NTT):
                nc.tensor.matmul(
                    sc_ps[bi][tt][:],
                    lhsT=htT[bi][:, lt, tt * P : (tt + 1) * P],
                    rhs=ks[bi][lt][:],
                    start=(lt == 0),
                    stop=False,
                )
            del ks[bi][lt]

        def s_mask(bi):
            for tt in range(NTT):
                # fold the mask penalty in as a K=1 broadcast accumulation
                nc.tensor.matmul(
                    sc_ps[bi][tt][:],
                    lhsT=ones_f[:],
                    rhs=pen_row[bi][:],
                    start=False,
                    stop=True,
                )

        def softmax_chains(bi):
            """Vector/Scalar-only part of the masked softmax."""
            sc = sc_ps[bi]
            negms = []
            for tt in range(NTT):
                negm = stats.tile([P, 1], F32, name="negm")
                nc.vector.reduce_max(
                    out=negm[:], in_=sc[tt][:], axis=AX.X, negate=True
                )
                negms.append(negm)
            abfs = []
            for tt in range(NTT):
                d = stats.tile([P, 1], F32, name="d")
                # exp in place in the score PSUM bank (saves an SBUF tile)
                nc.scalar.activation(
                    out=sc[tt][:], in_=sc[tt][:], func=ACT.Exp,
                    bias=negms[tt][:], scale=1.0, accum_out=d[:],
                )
                dr = stats.tile([P, 1], F32, name="dr")
                nc.vector.reciprocal(dr[:], d[:])
                abf = abfp.tile([P, TS], BF16, name="abf")
                nc.vector.tensor_scalar(abf[:], sc[tt][:], dr[:], None, ALU.mult)
                abfs.append(abf)
            return abfs

        def a_transpose(abfs):
            aT = onep.tile([P, NST, TT], BF16, name="aT")
            for tt in range(NTT):
                tpb = ptr.tile([P, 4, P], BF16, name="tp")
                for st in range(NST):
                    nc.tensor.transpose(
                        tpb[:, st, :], abfs[tt][:, st * P : (st + 1) * P], ident_bf[:]
                    )
                nc.vector.tensor_copy(aT[:, :, tt * P : (tt + 1) * P], tpb[:])
            return aT

        def ctx_mm(bi, aT):
            cT_bf = onep.tile([P, KT, TT], BF16, name="cT")
            for kt in range(KT):
                c_ps = pmm.tile([P, TT], F32, name="mm_ps")
                for st in range(NST):
                    nc.tensor.matmul(
                        c_ps[:],
                        lhsT=hs_bf[bi][:, st, kt * P : (kt + 1) * P],
                        rhs=aT[:, st, :],
                        start=(st == 0),
                        stop=(st == NST - 1),
                    )
                nc.vector.tensor_copy(cT_bf[:, kt, :], c_ps[:])
            return cT_bf

        def out_group(bi, cT_bf, tt, oc):
            o_ps = pmm.tile([P, OCH], F32, name="mm_ps")
            for kt in range(KT):
                nc.tensor.matmul(
                    o_ps[:],
                    lhsT=cT_bf[:, kt, tt * P : (tt + 1) * P],
                    rhs=wc_bf[:, kt, oc * OCH : (oc + 1) * OCH],
                    start=(kt == 0),
                    stop=False,
                )
            for kt in range(KT):
                nc.tensor.matmul(
                    o_ps[:],
                    lhsT=htT_bf[bi][:, kt, tt * P : (tt + 1) * P],
                    rhs=wc_bf[:, KT + kt, oc * OCH : (oc + 1) * OCH],
                    start=False,
                    stop=(not use_bias and kt == KT - 1),
                )
            if use_bias:
                nc.tensor.matmul(
                    o_ps[:],
                    lhsT=ones_f[:],
                    rhs=b_r[:, oc * OCH : (oc + 1) * OCH],
                    start=False,
                    stop=True,
                )
            ot = outp.tile([P, OCH], F32, name="out_t")
            nc.scalar.activation(out=ot[:], in_=o_ps[:], func=ACT.Tanh)
            nc.sync.dma_start(
                out=out_d[bi, tt * P : (tt + 1) * P, oc * OCH : (oc + 1) * OCH],
                in_=ot[:],
            )

        # ---------------- pipelined schedule over the 2 batches ----------
        htT[0] = htp.tile([P, KT, TT], F32R, name="htT")
        htT[1] = htp.tile([P, KT, TT], F32R, name="htT")
        htT_bf[0] = htbfp.tile([P, KT, TT], BF16, name="htT_bf")
        htT_bf[1] = htbfp.tile([P, KT, TT], BF16, name="htT_bf")

        # b0 input transposes fill the PE until W_a lands
        t_hs(0)
        for tt in range(NTT):
            t_ht_tile(0, tt)

        # b0 keys+score, software pipelined (score partials lag one step
        # behind the k-groups so keysT never fully materializes)
        sc_ps[0] = [psc.tile([P, TS], F32, name="sc_ps") for _ in range(NTT)]
        k_group(0, 0)
        for lt in range(1, KT):
            k_group(0, lt)
            s_partial(0, lt - 1)
        s_partial(0, KT - 1)
        s_mask(0)

        # b0 softmax on V/S; PE stays busy with b1's ht/hs transposes and
        # then b1 keys k-groups (keysp has 3 slots)
        abfs0 = softmax_chains(0)
        for tt in range(NTT):
            t_ht_tile(1, tt)
        t_hs(1)
        for lt in range(3):
            k_group(1, lt)
        aT0 = a_transpose(abfs0)
        cT0 = ctx_mm(0, aT0)

        # b1 remaining keys + score partials interleaved
        sc_ps[1] = [psc.tile([P, TS], F32, name="sc_ps") for _ in range(NTT)]
        for lt in range(KT):
            if lt + 3 < KT:
                s_partial(1, lt)
                k_group(1, lt + 3)
            else:
                s_partial(1, lt)
        s_mask(1)

        # b1 softmax (vector/scalar) overlaps b0's out projection
        abfs1 = softmax_chains(1)
        og = [(tt, oc) for tt in range(NTT) for oc in range(NOC)]
        for tt, oc in og[:3]:
            out_group(0, cT0, tt, oc)
        aT1 = a_transpose(abfs1)
        for tt, oc in og[3:]:
            out_group(0, cT0, tt, oc)
        cT1 = ctx_mm(1, aT1)
        for tt, oc in og:
            out_group(1, cT1, tt, oc)

    return nc


def _to_bf16(a: np.ndarray) -> np.ndarray:
    import ml_dtypes

    return np.ascontiguousarray(a.astype(ml_dtypes.bfloat16))


def make_in_maps(ht, hs, source, W_a, W_c, b):
    ht = np.ascontiguousarray(ht, dtype=np.float32)
    hs = np.ascontiguousarray(hs, dtype=np.float32)
    source = np.ascontiguousarray(source, dtype=np.int32)
    W_a = np.ascontiguousarray(W_a, dtype=np.float32)
    W_c_bf = _to_bf16(np.asarray(W_c, dtype=np.float32))
    b = np.ascontiguousarray(b, dtype=np.float32)
    # mask penalty rows: 0 at valid (prefix) positions, -1e9 at padding
    lens = (source != 0).sum(axis=1)
    pen = np.where(
        np.arange(TS, dtype=np.int64)[None, :] < lens[:, None], 0.0, -1e9
    ).astype(np.float32)
    in_maps = []
    for c in range(NCORES):
        sl = slice(c * BL, (c + 1) * BL)
        in_maps.append(
            {
                "ht": ht[sl],
                "hs": hs[sl],
                "pen": pen[sl],
                "W_a": W_a,
                "W_c": W_c_bf,
                "b": b,
            }
        )
    return in_maps


_NC_CACHE: dict = {}


def _get_nc(use_bias: bool = False):
    key = f"nc_bias{use_bias}"
    if key not in _NC_CACHE:
        nc = build_core(use_bias=use_bias)
        if not nc.is_finalized():
            nc.finalize()
        _NC_CACHE[key] = nc
    return _NC_CACHE[key]


def run_on_hw(ht, hs, source, W_a, W_c, b, trace=False, **kw):
    nc = _get_nc(use_bias=bool(np.any(np.asarray(b) != 0)))
    in_maps = make_in_maps(ht, hs, source, W_a, W_c, b)
    res = run_bass_kernel_spmd(nc, in_maps, core_ids=list(range(NCORES)), trace=trace, **kw)
    out = np.concatenate([res.results[c]["out"] for c in range(NCORES)], axis=0)
    return out, res


def kernel(ht, hs, source, W_a, W_c, b):
    out, _ = run_on_hw(ht, hs, source, W_a, W_c, b, trace=False)
    return out
